# revision 7
# baseline (speedup 1.0000x reference)
# Multi-head causal attention (B=4, T=2048, D=1024, H=16) on 8 TRN2 NeuronCores.
#
# Sharding: data-parallel over the 4 batches x 2-way split of query rows
# (interleaved 128-row blocks for causal load balance). Every core computes
# K/V for its batch's full sequence (replicated within the batch pair), so
# there are NO collectives. Each core produces the output rows for its own
# 1024 query tokens; the host reassembles.
#
# Per-core kernel (all matmul operands bf16, fp32 PSUM accumulation):
#   qT/kT = W @ x^T (per head-pair group), V kept (t, d)-major with an extra
#   ones column so the attention-value matmul also produces the softmax
#   denominator. Scores are computed transposed (tk partition, tq free),
#   exp on the Scalar engine (no max subtraction: |scores| <= ~3), causal
#   masking via multiplicative {0,1} mask tiles fed per-core, denominator
#   broadcast across partitions on GpSimd.
import numpy as np
import ml_dtypes

B, T, D, H, DH, P = 4, 2048, 1024, 16, 64, 128
NQ = 1024          # query tokens per core
NCORES = 8
BF16 = ml_dtypes.bfloat16

_COMPILED = {}


def _build_nc():
    from contextlib import ExitStack
    import concourse.mybir as mybir
    import concourse.tile as tile
    from concourse import bacc

    bf = mybir.dt.bfloat16
    f32 = mybir.dt.float32
    EXP = mybir.ActivationFunctionType.Exp

    nc = bacc.Bacc("TRN2", target_bir_lowering=False, debug=False,
                   num_devices=NCORES)

    # ---- DRAM I/O ----
    xk_d = nc.dram_tensor("xk", [D, T], bf, kind="ExternalInput").ap()
    xq_d = nc.dram_tensor("xq", [D, NQ], bf, kind="ExternalInput").ap()
    wq_d = nc.dram_tensor("wqT", [D, D], bf, kind="ExternalInput").ap()
    wk_d = nc.dram_tensor("wkT", [D, D], bf, kind="ExternalInput").ap()
    wv_d = nc.dram_tensor("wvT", [D, D], bf, kind="ExternalInput").ap()
    wo_d = nc.dram_tensor("woT", [D, D], bf, kind="ExternalInput").ap()
    bq_d = nc.dram_tensor("bq_r", [P, 8], f32, kind="ExternalInput").ap()
    bk_d = nc.dram_tensor("bk_r", [P, 8], f32, kind="ExternalInput").ap()
    bo_d = nc.dram_tensor("bo_r", [P, 8], f32, kind="ExternalInput").ap()
    bv_d = nc.dram_tensor("bv_bc", [P, D], bf, kind="ExternalInput").ap()
    cm_d = nc.dram_tensor("cmask", [4, P, 256], bf, kind="ExternalInput").ap()
    y_d = nc.dram_tensor("yT", [D, NQ], f32, kind="ExternalOutput").ap()

    xk_r = xk_d.rearrange("(g p) t -> p g t", p=P)
    xq_r = xq_d.rearrange("(g p) t -> p g t", p=P)

    with tile.TileContext(nc) as tc, ExitStack() as ctx:
        const = ctx.enter_context(tc.tile_pool(name="const", bufs=1))
        xchunk = ctx.enter_context(tc.tile_pool(name="xchunk", bufs=2))
        expps = ctx.enter_context(tc.tile_pool(name="expps", bufs=3))
        small = ctx.enter_context(tc.tile_pool(name="small", bufs=2))
        ps_s = ctx.enter_context(tc.tile_pool(name="ps_s", bufs=2, space="PSUM"))
        ps_av = ctx.enter_context(tc.tile_pool(name="ps_av", bufs=1, space="PSUM"))
        ps_m = ctx.enter_context(tc.tile_pool(name="ps_m", bufs=2, space="PSUM"))

        # ---- resident SBUF tensors ----
        wq_sb = const.tile([P, 8, 8, P], bf)     # [k, kg, g, dout]
        wk_sb = const.tile([P, 8, 8, P], bf)
        wv_sb = const.tile([P, 8, D], bf)        # [k, kg, d] (moving operand)
        wo_sb = const.tile([P, 8, 8, P], bf)     # [d, g, o, dout]
        k_sb = const.tile([P, 8, T], bf)         # k^T per head-pair group
        q_sb = const.tile([P, 8, NQ], bf)
        v1_sb = const.tile([P, 16, 16, 65], bf)  # [tk, tkgrp, head, V|1]
        a_sb = const.tile([P, 8, NQ], bf)        # attention out (d, tq)
        mk_sb = const.tile([P, 4, 256], bf)
        bq_sb = const.tile([P, 8], f32)
        bk_sb = const.tile([P, 8], f32)
        bo_sb = const.tile([P, 8], f32)
        bv_sb = const.tile([P, D], bf)

        dma = nc.sync.dma_start
        dma(out=wv_sb, in_=wv_d.rearrange("(kg p) d -> p kg d", p=P))
        dma(out=wk_sb, in_=wk_d.rearrange("(kg p) (g c) -> p kg g c", p=P, c=P))
        dma(out=wq_sb, in_=wq_d.rearrange("(kg p) (g c) -> p kg g c", p=P, c=P))
        dma(out=wo_sb, in_=wo_d.rearrange("(g p) (o c) -> p g o c", p=P, c=P))
        dma(out=mk_sb, in_=cm_d.rearrange("m p c -> p m c"))
        dma(out=bq_sb, in_=bq_d)
        dma(out=bk_sb, in_=bk_d)
        dma(out=bo_sb, in_=bo_d)
        dma(out=bv_sb, in_=bv_d)

        nc.vector.memset(v1_sb[:, :, :, 64:65], 1.0)

        mm = nc.tensor.matmul

        def proj_chunk(ts):
            # stream 512 tokens of x^T, project into V (+bias, ones layout)
            # and K for all 8 head-pair groups
            xc = xchunk.tile([P, 8, 512], bf, name="xc", tag="xc")
            dma(out=xc, in_=xk_r[:, :, ts * 512:(ts + 1) * 512])
            for dhalf in range(2):
                for ti in range(4):
                    t = 4 * ts + ti
                    ps = ps_m.tile([P, 512], f32, name="ps", tag="ps")
                    for kg in range(8):
                        mm(ps, lhsT=xc[:, kg, ti * P:(ti + 1) * P],
                           rhs=wv_sb[:, kg, dhalf * 512:(dhalf + 1) * 512],
                           start=(kg == 0), stop=(kg == 7))
                    nc.vector.tensor_add(
                        out=v1_sb[:, t, 8 * dhalf:8 * dhalf + 8, 0:64],
                        in0=ps.rearrange("p (h c) -> p h c", c=64),
                        in1=bv_sb[:, dhalf * 512:(dhalf + 1) * 512]
                            .rearrange("p (h c) -> p h c", c=64))
            for g in range(8):
                ps = ps_m.tile([P, 512], f32, name="ps", tag="ps")
                for kg in range(8):
                    mm(ps, lhsT=wk_sb[:, kg, g, :], rhs=xc[:, kg, :],
                       start=(kg == 0), stop=(kg == 7))
                nc.vector.tensor_scalar_add(
                    out=k_sb[:, g, ts * 512:(ts + 1) * 512], in0=ps,
                    scalar1=bk_sb[:, g:g + 1])

        def proj_q_chunk(ts):
            xc = xchunk.tile([P, 8, 512], bf, name="xc", tag="xc")
            dma(out=xc, in_=xq_r[:, :, ts * 512:(ts + 1) * 512])
            for g in range(8):
                ps = ps_m.tile([P, 512], f32, name="ps", tag="ps")
                for kg in range(8):
                    mm(ps, lhsT=wq_sb[:, kg, g, :], rhs=xc[:, kg, :],
                       start=(kg == 0), stop=(kg == 7))
                nc.vector.tensor_scalar_add(
                    out=q_sb[:, g, ts * 512:(ts + 1) * 512], in0=ps,
                    scalar1=bq_sb[:, g:g + 1])

        def attn_slot(g, j):
            pav = [ps_av.tile([65, 256], f32, tag=f"pav{c}",
                              name=f"pav{c}") for c in (0, 1)]
            last_k = 4 * j + 3
            for kk in range(2 * j + 2):  # pairs of tk blocks
                kb = (2 * kk, 2 * kk + 1)
                ps = ps_s.tile([P, 1024], f32, name="scps", tag="scps")
                expp = expps.tile([P, 1024], bf, name="expp", tag="expp")
                for c in (0, 1):         # head within pair
                    for i in (0, 1):     # tk block within pair
                        mm(ps[:, c * 512 + i * 256: c * 512 + i * 256 + 256],
                           lhsT=k_sb[64 * c:64 * c + 64, g,
                                     kb[i] * P:(kb[i] + 1) * P],
                           rhs=q_sb[64 * c:64 * c + 64, g,
                                    j * 256:(j + 1) * 256],
                           start=True, stop=True,
                           tile_position=(64 * c, 0))
                nc.scalar.activation(out=expp, in_=ps, func=EXP, scale=0.125)
                for i in (0, 1):
                    m = kb[i] - 4 * j
                    if 0 <= m <= 3:
                        for c in (0, 1):
                            sl = expp[:, c * 512 + i * 256:
                                      c * 512 + i * 256 + 256]
                            nc.vector.tensor_mul(sl, sl, mk_sb[:, m, :])
                for c in (0, 1):
                    for i in (0, 1):
                        mm(pav[c],
                           lhsT=v1_sb[:, kb[i], 2 * g + c, :],
                           rhs=expp[:, c * 512 + i * 256:
                                    c * 512 + i * 256 + 256],
                           start=(kb[i] == 0), stop=(kb[i] == last_k))
            for c in (0, 1):
                rec = small.tile([1, 256], f32, tag="rec", name="rec")
                nc.vector.reciprocal(out=rec, in_=pav[c][64:65, :])
                sbb = small.tile([64, 256], f32, tag="sbb", name="sbb")
                nc.gpsimd.partition_broadcast(sbb, rec)
                nc.vector.tensor_mul(
                    out=a_sb[64 * c:64 * c + 64, g, j * 256:(j + 1) * 256],
                    in0=pav[c][0:64, :], in1=sbb)

        # chunk-major projections interleaved with attention by query slot:
        # attn slot j needs K/V token chunks 0..j and Q chunk j//2
        proj_chunk(0)
        proj_q_chunk(0)
        proj_chunk(1)
        for g in range(8):
            attn_slot(g, 0)
        proj_q_chunk(1)
        proj_chunk(2)
        for g in range(8):
            attn_slot(g, 1)
        proj_chunk(3)
        for g in range(8):
            attn_slot(g, 2)
        for g in range(8):
            attn_slot(g, 3)

        # ---- output projection ----
        for ts in range(2):
            for o in range(8):
                ps = ps_m.tile([P, 512], f32, name="ps", tag="ps")
                for g in range(8):
                    mm(ps, lhsT=wo_sb[:, g, o, :],
                       rhs=a_sb[:, g, ts * 512:(ts + 1) * 512],
                       start=(g == 0), stop=(g == 7))
                ysb = small.tile([P, 512], f32, tag="ysb", name="ysb")
                nc.vector.tensor_scalar_add(out=ysb, in0=ps,
                                            scalar1=bo_sb[:, o:o + 1])
                dma(out=y_d[o * P:(o + 1) * P, ts * 512:(ts + 1) * 512],
                    in_=ysb)

    nc.compile()
    return nc


def _get_nc():
    if "nc" not in _COMPILED:
        _COMPILED["nc"] = _build_nc()
    return _COMPILED["nc"]


def _core_token_blocks(par):
    return [2 * i + par for i in range(8)]


def _masks(par):
    tri = np.triu(np.ones((P, P), np.float32))   # keep tk <= tq
    on = np.ones((P, P), np.float32)
    z = np.zeros((P, P), np.float32)
    if par == 0:
        rows = [[tri, on], [z, on], [z, tri], [z, z]]
    else:
        rows = [[on, on], [tri, on], [z, on], [z, tri]]
    return np.stack([np.concatenate(r, axis=1) for r in rows]).astype(BF16)


def _make_in_maps(x, wq, bq, wk, bk, wv, bv, wo, bo):
    bfc = lambda a: np.ascontiguousarray(np.asarray(a, np.float32).T).astype(BF16)
    shared = {
        "wqT": bfc(wq), "wkT": bfc(wk), "wvT": bfc(wv), "woT": bfc(wo),
        "bq_r": np.ascontiguousarray(np.asarray(bq, np.float32).reshape(8, P).T),
        "bk_r": np.ascontiguousarray(np.asarray(bk, np.float32).reshape(8, P).T),
        "bo_r": np.ascontiguousarray(np.asarray(bo, np.float32).reshape(8, P).T),
        "bv_bc": np.ascontiguousarray(
            np.broadcast_to(np.asarray(bv, np.float32).astype(BF16), (P, D))),
    }
    masks = [_masks(0), _masks(1)]
    in_maps, idx_list = [], []
    for core in range(NCORES):
        b, par = core // 2, core % 2
        blocks = _core_token_blocks(par)
        idx = np.concatenate([np.arange(P * blk, P * blk + P) for blk in blocks])
        xT = np.asarray(x[b], np.float32).T
        m = dict(shared)
        m["xk"] = np.ascontiguousarray(xT).astype(BF16)
        m["xq"] = np.ascontiguousarray(xT[:, idx]).astype(BF16)
        m["cmask"] = masks[par]
        in_maps.append(m)
        idx_list.append((b, idx))
    return in_maps, idx_list


def _run(inputs, trace=False):
    from concourse.bass_utils import run_bass_kernel_spmd
    nc = _get_nc()
    in_maps, idx_list = _make_in_maps(**inputs)
    res = run_bass_kernel_spmd(nc, in_maps, list(range(NCORES)), trace=trace)
    y = np.empty((B, T, D), np.float32)
    for core in range(NCORES):
        b, idx = idx_list[core]
        y[b][idx, :] = res.results[core]["yT"].T
    return y, res


def kernel(**inputs):
    y, _ = _run(inputs, trace=False)
    return y


# revision 22
# speedup vs baseline: 1.5545x; 1.5545x over previous
# Multi-head causal attention (B=4, T=2048, D=1024, H=16) on 8 TRN2 NeuronCores.
#
# Sharding: data-parallel over the 4 batches x 2-way split of query rows
# (interleaved 128-row blocks for causal load balance). Every core computes
# K/V for its batch's full sequence (replicated within the batch pair), so
# there are NO collectives. Each core produces the output rows for its own
# 1024 query tokens; the host reassembles.
#
# Per-core kernel (all matmul operands bf16, fp32 PSUM accumulation):
#   qT/kT = W @ x^T (per head-pair group), V kept (t, d)-major with an extra
#   ones column so the attention-value matmul also produces the softmax
#   denominator. Scores are computed transposed (tk partition, tq free),
#   exp on the Scalar engine (no max subtraction: |scores| <= ~3), causal
#   masking via multiplicative {0,1} mask tiles fed per-core, denominator
#   broadcast across partitions on GpSimd.
import numpy as np
import ml_dtypes

B, T, D, H, DH, P = 4, 2048, 1024, 16, 64, 128
NQ = 1024          # query tokens per core
NCORES = 8
BF16 = ml_dtypes.bfloat16

_COMPILED = {}


def _build_nc():
    from contextlib import ExitStack
    import concourse.mybir as mybir
    import concourse.tile as tile
    from concourse import bacc

    bf = mybir.dt.bfloat16
    f32 = mybir.dt.float32
    EXP = mybir.ActivationFunctionType.Exp

    nc = bacc.Bacc("TRN2", target_bir_lowering=False, debug=False,
                   num_devices=NCORES)

    # ---- DRAM I/O ----
    xk_d = nc.dram_tensor("xk", [D, T], bf, kind="ExternalInput").ap()
    xq_d = nc.dram_tensor("xq", [D, NQ], bf, kind="ExternalInput").ap()
    wq_d = nc.dram_tensor("wqT", [D, D], bf, kind="ExternalInput").ap()
    wk_d = nc.dram_tensor("wkT", [D, D], bf, kind="ExternalInput").ap()
    wv_d = nc.dram_tensor("wvT", [D, D], bf, kind="ExternalInput").ap()
    wo_d = nc.dram_tensor("woT", [D, D], bf, kind="ExternalInput").ap()
    bq_d = nc.dram_tensor("bq_r", [P, 8], f32, kind="ExternalInput").ap()
    bk_d = nc.dram_tensor("bk_r", [P, 8], f32, kind="ExternalInput").ap()
    bo_d = nc.dram_tensor("bo_r", [P, 8], f32, kind="ExternalInput").ap()
    bv_d = nc.dram_tensor("bv_bc", [P, D], bf, kind="ExternalInput").ap()
    cm_d = nc.dram_tensor("cmask", [2, P, 1024], bf, kind="ExternalInput").ap()
    y_d = nc.dram_tensor("yT", [D, NQ], f32, kind="ExternalOutput").ap()

    xk_r = xk_d.rearrange("(g p) t -> p g t", p=P)
    xq_r = xq_d.rearrange("(g p) t -> p g t", p=P)

    with tile.TileContext(nc) as tc, ExitStack() as ctx:
        const = ctx.enter_context(tc.tile_pool(name="const", bufs=1))
        xchunk = ctx.enter_context(tc.tile_pool(name="xchunk", bufs=2))
        expps = ctx.enter_context(tc.tile_pool(name="expps", bufs=3))
        small = ctx.enter_context(tc.tile_pool(name="small", bufs=2))
        ps_s = ctx.enter_context(tc.tile_pool(name="ps_s", bufs=2, space="PSUM"))
        ps_av = ctx.enter_context(tc.tile_pool(name="ps_av", bufs=1, space="PSUM"))
        ps_m = ctx.enter_context(tc.tile_pool(name="ps_m", bufs=2, space="PSUM"))

        # ---- resident SBUF tensors ----
        wq_sb = const.tile([P, 8, 8, P], bf)     # [k, kg, g, dout]
        wk_sb = const.tile([P, 8, 8, P], bf)
        wv_sb = const.tile([P, 8, D], bf)        # [k, kg, d] (moving operand)
        wo_sb = const.tile([P, 8, 8, P], bf)     # [d, g, o, dout]
        k_sb = const.tile([P, 8, T], bf)         # k^T per head-pair group
        q_sb = const.tile([P, 8, NQ], bf)
        v1_sb = const.tile([P, 16, 16, 65], bf)  # [tk, tkgrp, head, V|1]
        a_sb = const.tile([P, 8, NQ], bf)        # attention out (d, tq)
        mk_sb = const.tile([P, 2, 1024], bf)
        bq_sb = const.tile([P, 8], f32)
        bk_sb = const.tile([P, 8], f32)
        bo_sb = const.tile([P, 8], f32)
        bv_sb = const.tile([P, D], bf)

        dma = nc.sync.dma_start
        dma(out=wv_sb, in_=wv_d.rearrange("(kg p) d -> p kg d", p=P))
        dma(out=wk_sb, in_=wk_d.rearrange("(kg p) (g c) -> p kg g c", p=P, c=P))
        dma(out=wq_sb, in_=wq_d.rearrange("(kg p) (g c) -> p kg g c", p=P, c=P))
        dma(out=wo_sb, in_=wo_d.rearrange("(g p) (o c) -> p g o c", p=P, c=P))
        dma(out=mk_sb, in_=cm_d.rearrange("m p c -> p m c"))
        dma(out=bq_sb, in_=bq_d)
        dma(out=bk_sb, in_=bk_d)
        dma(out=bo_sb, in_=bo_d)
        dma(out=bv_sb, in_=bv_d)

        nc.vector.memset(v1_sb[:, :, :, 64:65], 1.0)

        mm = nc.tensor.matmul

        def proj_chunk(ts):
            # stream 512 tokens of x^T, project into V (+bias, ones layout)
            # and K for all 8 head-pair groups
            xc = xchunk.tile([P, 8, 512], bf, name="xc", tag="xc")
            dma(out=xc, in_=xk_r[:, :, ts * 512:(ts + 1) * 512])
            for dhalf in range(2):
                for ti in range(4):
                    t = 4 * ts + ti
                    ps = ps_m.tile([P, 512], f32, name="ps", tag="ps")
                    for kg in range(8):
                        mm(ps, lhsT=xc[:, kg, ti * P:(ti + 1) * P],
                           rhs=wv_sb[:, kg, dhalf * 512:(dhalf + 1) * 512],
                           start=(kg == 0), stop=(kg == 7))
                    nc.vector.tensor_add(
                        out=v1_sb[:, t, 8 * dhalf:8 * dhalf + 8, 0:64],
                        in0=ps.rearrange("p (h c) -> p h c", c=64),
                        in1=bv_sb[:, dhalf * 512:(dhalf + 1) * 512]
                            .rearrange("p (h c) -> p h c", c=64))
            for g in range(8):
                ps = ps_m.tile([P, 512], f32, name="ps", tag="ps")
                for kg in range(8):
                    mm(ps, lhsT=wk_sb[:, kg, g, :], rhs=xc[:, kg, :],
                       start=(kg == 0), stop=(kg == 7))
                nc.vector.tensor_scalar_add(
                    out=k_sb[:, g, ts * 512:(ts + 1) * 512], in0=ps,
                    scalar1=bk_sb[:, g:g + 1])

        def proj_q_chunk(ts):
            xc = xchunk.tile([P, 8, 512], bf, name="xc", tag="xc")
            dma(out=xc, in_=xq_r[:, :, ts * 512:(ts + 1) * 512])
            for g in range(8):
                ps = ps_m.tile([P, 512], f32, name="ps", tag="ps")
                for kg in range(8):
                    mm(ps, lhsT=wq_sb[:, kg, g, :], rhs=xc[:, kg, :],
                       start=(kg == 0), stop=(kg == 7))
                nc.vector.tensor_scalar_add(
                    out=q_sb[:, g, ts * 512:(ts + 1) * 512], in0=ps,
                    scalar1=bq_sb[:, g:g + 1])

        def attn_slot(g, j):
            # one accumulator bank PER HEAD: PSUM accumulation groups must be
            # bank-aligned on HW (a group at a 256-col offset inside a bank
            # silently corrupts). Row 64 collects the softmax denominator
            # via the ones column of v1_sb.
            pav = [ps_av.tile([65, 256], f32, tag=f"pav{c}",
                              name=f"pav{c}") for c in (0, 1)]
            last_k = 4 * j + 3
            for kk in range(2 * j + 2):  # pairs of tk blocks
                kb = (2 * kk, 2 * kk + 1)
                ps = ps_s.tile([P, 1024], f32, name="scps", tag="scps")
                expp = expps.tile([P, 1024], bf, name="expp", tag="expp")
                for c in (0, 1):         # head within pair
                    for i in (0, 1):     # tk block within pair
                        mm(ps[:, c * 512 + i * 256: c * 512 + i * 256 + 256],
                           lhsT=k_sb[64 * c:64 * c + 64, g,
                                     kb[i] * P:(kb[i] + 1) * P],
                           rhs=q_sb[64 * c:64 * c + 64, g,
                                    j * 256:(j + 1) * 256],
                           start=True, stop=True,
                           tile_position=(64 * c, 0))
                nc.scalar.activation(out=expp, in_=ps, func=EXP, scale=0.125)
                if kk >= 2 * j:
                    nc.vector.tensor_mul(expp, expp, mk_sb[:, kk - 2 * j, :])
                for c in (0, 1):
                    for i in (0, 1):
                        mm(pav[c],
                           lhsT=v1_sb[:, kb[i], 2 * g + c, :],
                           rhs=expp[:, c * 512 + i * 256:
                                    c * 512 + i * 256 + 256],
                           start=(kb[i] == 0), stop=(kb[i] == last_k))
            # copy accumulators to SBUF right away so the PSUM banks free up
            # for the next slot; normalize runs off the PE critical path
            av = [small.tile([65, 256], f32, tag=f"av{c}", bufs=2,
                             name=f"av{c}") for c in (0, 1)]
            for c in (0, 1):
                nc.vector.tensor_copy(out=av[c], in_=pav[c])
            # both heads' denominators into one partition-base-0 tile
            # (reciprocal_approx_fast corrupts base!=0 inputs on HW)
            den2 = small.tile([1, 512], f32, tag="den2", bufs=1, name="den2")
            for c in (0, 1):
                nc.vector.tensor_copy(out=den2[:, c * 256:(c + 1) * 256],
                                      in_=av[c][64:65, :])
            rec = small.tile([1, 512], f32, tag="rec", bufs=1, name="rec")
            nc.vector.reciprocal_approx_fast(out=rec, in_=den2)
            sbb = small.tile([64, 512], f32, tag="sbb", name="sbb")
            nc.gpsimd.partition_broadcast(sbb, rec)
            for c in (0, 1):
                nc.vector.tensor_mul(
                    out=a_sb[64 * c:64 * c + 64, g, j * 256:(j + 1) * 256],
                    in0=av[c][0:64, :], in1=sbb[:, c * 256:(c + 1) * 256])

        # chunk-major projections interleaved with attention by query slot:
        # attn slot j needs K/V token chunks 0..j and Q chunk j//2
        proj_chunk(0)
        proj_q_chunk(0)
        proj_chunk(1)
        for g in range(8):
            attn_slot(g, 0)
        proj_q_chunk(1)
        proj_chunk(2)
        for g in range(8):
            attn_slot(g, 1)
        proj_chunk(3)
        for g in range(8):
            attn_slot(g, 2)
        for g in range(8):
            attn_slot(g, 3)

        # ---- output projection ----
        for ts in range(2):
            for o in range(8):
                ps = ps_m.tile([P, 512], f32, name="ps", tag="ps")
                for g in range(8):
                    mm(ps, lhsT=wo_sb[:, g, o, :],
                       rhs=a_sb[:, g, ts * 512:(ts + 1) * 512],
                       start=(g == 0), stop=(g == 7))
                ysb = small.tile([P, 512], f32, tag="ysb", name="ysb")
                nc.vector.tensor_scalar_add(out=ysb, in0=ps,
                                            scalar1=bo_sb[:, o:o + 1])
                dma(out=y_d[o * P:(o + 1) * P, ts * 512:(ts + 1) * 512],
                    in_=ysb)

    nc.compile()
    return nc


def _get_nc():
    if "nc" not in _COMPILED:
        _COMPILED["nc"] = _build_nc()
    return _COMPILED["nc"]


def _core_token_blocks(par):
    return [2 * i + par for i in range(8)]


def _masks(par):
    tri = np.triu(np.ones((P, P), np.float32))   # keep tk <= tq
    on = np.ones((P, P), np.float32)
    z = np.zeros((P, P), np.float32)
    if par == 0:
        m = [np.concatenate(r, axis=1)
             for r in [[tri, on], [z, on], [z, tri], [z, z]]]
    else:
        m = [np.concatenate(r, axis=1)
             for r in [[on, on], [tri, on], [z, on], [z, tri]]]
    # combined group masks matching expp layout [k h0 | k+1 h0 | k h1 | k+1 h1]
    row0 = np.concatenate([m[0], m[1], m[0], m[1]], axis=1)
    row1 = np.concatenate([m[2], m[3], m[2], m[3]], axis=1)
    return np.stack([row0, row1]).astype(BF16)


def _make_in_maps(x, wq, bq, wk, bk, wv, bv, wo, bo):
    bfc = lambda a: np.ascontiguousarray(np.asarray(a, np.float32).T).astype(BF16)
    shared = {
        "wqT": bfc(wq), "wkT": bfc(wk), "wvT": bfc(wv), "woT": bfc(wo),
        "bq_r": np.ascontiguousarray(np.asarray(bq, np.float32).reshape(8, P).T),
        "bk_r": np.ascontiguousarray(np.asarray(bk, np.float32).reshape(8, P).T),
        "bo_r": np.ascontiguousarray(np.asarray(bo, np.float32).reshape(8, P).T),
        "bv_bc": np.ascontiguousarray(
            np.broadcast_to(np.asarray(bv, np.float32).astype(BF16), (P, D))),
    }
    masks = [_masks(0), _masks(1)]
    in_maps, idx_list = [], []
    for core in range(NCORES):
        b, par = core // 2, core % 2
        blocks = _core_token_blocks(par)
        idx = np.concatenate([np.arange(P * blk, P * blk + P) for blk in blocks])
        xT = np.asarray(x[b], np.float32).T
        m = dict(shared)
        m["xk"] = np.ascontiguousarray(xT).astype(BF16)
        m["xq"] = np.ascontiguousarray(xT[:, idx]).astype(BF16)
        m["cmask"] = masks[par]
        in_maps.append(m)
        idx_list.append((b, idx))
    return in_maps, idx_list


def _run(inputs, trace=False):
    from concourse.bass_utils import run_bass_kernel_spmd
    nc = _get_nc()
    in_maps, idx_list = _make_in_maps(**inputs)
    res = run_bass_kernel_spmd(nc, in_maps, list(range(NCORES)), trace=trace)
    y = np.empty((B, T, D), np.float32)
    for core in range(NCORES):
        b, idx = idx_list[core]
        y[b][idx, :] = res.results[core]["yT"].T
    return y, res


def kernel(**inputs):
    y, _ = _run(inputs, trace=False)
    return y


# revision 25
# speedup vs baseline: 1.6492x; 1.0609x over previous
# Multi-head causal attention (B=4, T=2048, D=1024, H=16) on 8 TRN2 NeuronCores.
#
# Sharding: data-parallel over the 4 batches x 2-way split of query rows
# (interleaved 128-row blocks for causal load balance). Every core computes
# K/V for its batch's full sequence (replicated within the batch pair), so
# there are NO collectives. Each core produces the output rows for its own
# 1024 query tokens; the host reassembles.
#
# Per-core kernel (all matmul operands bf16, fp32 PSUM accumulation):
#   qT/kT = W @ x^T (per head-pair group), V kept (t, d)-major with an extra
#   ones column so the attention-value matmul also produces the softmax
#   denominator. Scores are computed transposed (tk partition, tq free),
#   exp on the Scalar engine (no max subtraction: |scores| <= ~3), causal
#   masking via multiplicative {0,1} mask tiles fed per-core, denominator
#   broadcast across partitions on GpSimd.
import numpy as np
import ml_dtypes

B, T, D, H, DH, P = 4, 2048, 1024, 16, 64, 128
NQ = 1024          # query tokens per core
NCORES = 8
BF16 = ml_dtypes.bfloat16

_COMPILED = {}


def _build_nc():
    from contextlib import ExitStack
    import concourse.mybir as mybir
    import concourse.tile as tile
    from concourse import bacc

    bf = mybir.dt.bfloat16
    f32 = mybir.dt.float32
    EXP = mybir.ActivationFunctionType.Exp

    nc = bacc.Bacc("TRN2", target_bir_lowering=False, debug=False,
                   num_devices=NCORES)

    # ---- DRAM I/O ----
    xk_d = nc.dram_tensor("xk", [D, T], bf, kind="ExternalInput").ap()
    xq_d = nc.dram_tensor("xq", [D, NQ], bf, kind="ExternalInput").ap()
    wq_d = nc.dram_tensor("wqT", [D, D], bf, kind="ExternalInput").ap()
    wk_d = nc.dram_tensor("wkT", [D, D], bf, kind="ExternalInput").ap()
    wv_d = nc.dram_tensor("wvT", [D, D], bf, kind="ExternalInput").ap()
    wo_d = nc.dram_tensor("woT", [D, D], bf, kind="ExternalInput").ap()
    bq_d = nc.dram_tensor("bq_r", [P, 8], f32, kind="ExternalInput").ap()
    bk_d = nc.dram_tensor("bk_r", [P, 8], f32, kind="ExternalInput").ap()
    bo_d = nc.dram_tensor("bo_r", [P, 8], f32, kind="ExternalInput").ap()
    bv_d = nc.dram_tensor("bv_bc", [P, D], bf, kind="ExternalInput").ap()
    cm_d = nc.dram_tensor("cmask", [2, P, 1024], bf, kind="ExternalInput").ap()
    y_d = nc.dram_tensor("yT", [D, NQ], f32, kind="ExternalOutput").ap()

    xk_r = xk_d.rearrange("(g p) t -> p g t", p=P)
    xq_r = xq_d.rearrange("(g p) t -> p g t", p=P)

    with tile.TileContext(nc) as tc, ExitStack() as ctx:
        const = ctx.enter_context(tc.tile_pool(name="const", bufs=1))
        xchunk = ctx.enter_context(tc.tile_pool(name="xchunk", bufs=2))
        expps = ctx.enter_context(tc.tile_pool(name="expps", bufs=4))
        small = ctx.enter_context(tc.tile_pool(name="small", bufs=2))
        ps_s = ctx.enter_context(tc.tile_pool(name="ps_s", bufs=2, space="PSUM"))
        ps_av = ctx.enter_context(tc.tile_pool(name="ps_av", bufs=1, space="PSUM"))
        ps_m = ctx.enter_context(tc.tile_pool(name="ps_m", bufs=2, space="PSUM"))

        # ---- resident SBUF tensors ----
        wq_sb = const.tile([P, 8, 8, P], bf)     # [k, kg, g, dout]
        wk_sb = const.tile([P, 8, 8, P], bf)
        wv_sb = const.tile([P, 8, D], bf)        # [k, kg, d] (moving operand)
        wo_sb = const.tile([P, 8, 8, P], bf)     # [d, g, o, dout]
        k_sb = const.tile([P, 8, T], bf)         # k^T per head-pair group
        q_sb = const.tile([P, 8, NQ], bf)
        v1_sb = const.tile([P, 16, 16, 65], bf)  # [tk, tkgrp, head, V|1]
        a_sb = const.tile([P, 8, NQ], bf)        # attention out (d, tq)
        mk_sb = const.tile([P, 2, 1024], bf)
        bq_sb = const.tile([P, 8], f32)
        bk_sb = const.tile([P, 8], f32)
        bo_sb = const.tile([P, 8], f32)
        bv_sb = const.tile([P, D], bf)

        dma = nc.sync.dma_start
        dma(out=wv_sb, in_=wv_d.rearrange("(kg p) d -> p kg d", p=P))
        dma(out=wk_sb, in_=wk_d.rearrange("(kg p) (g c) -> p kg g c", p=P, c=P))
        dma(out=wq_sb, in_=wq_d.rearrange("(kg p) (g c) -> p kg g c", p=P, c=P))
        dma(out=wo_sb, in_=wo_d.rearrange("(g p) (o c) -> p g o c", p=P, c=P))
        dma(out=mk_sb, in_=cm_d.rearrange("m p c -> p m c"))
        dma(out=bq_sb, in_=bq_d)
        dma(out=bk_sb, in_=bk_d)
        dma(out=bo_sb, in_=bo_d)
        dma(out=bv_sb, in_=bv_d)

        nc.vector.memset(v1_sb[:, :, :, 64:65], 1.0)

        mm = nc.tensor.matmul

        def _v_group(xc, ts, dhalf, ti):
            t = 4 * ts + ti
            ps = ps_m.tile([P, 512], f32, name="ps", tag="ps")
            for kg in range(8):
                mm(ps, lhsT=xc[:, kg, ti * P:(ti + 1) * P],
                   rhs=wv_sb[:, kg, dhalf * 512:(dhalf + 1) * 512],
                   start=(kg == 0), stop=(kg == 7))
            nc.vector.tensor_add(
                out=v1_sb[:, t, 8 * dhalf:8 * dhalf + 8, 0:64],
                in0=ps.rearrange("p (h c) -> p h c", c=64),
                in1=bv_sb[:, dhalf * 512:(dhalf + 1) * 512]
                    .rearrange("p (h c) -> p h c", c=64))

        def _k_group(xc, ts, g):
            ps = ps_m.tile([P, 512], f32, name="ps", tag="ps")
            for kg in range(8):
                mm(ps, lhsT=wk_sb[:, kg, g, :], rhs=xc[:, kg, :],
                   start=(kg == 0), stop=(kg == 7))
            nc.vector.tensor_scalar_add(
                out=k_sb[:, g, ts * 512:(ts + 1) * 512], in0=ps,
                scalar1=bk_sb[:, g:g + 1])

        def _q_group(xc, ts, g):
            ps = ps_m.tile([P, 512], f32, name="ps", tag="ps")
            for kg in range(8):
                mm(ps, lhsT=wq_sb[:, kg, g, :], rhs=xc[:, kg, :],
                   start=(kg == 0), stop=(kg == 7))
            nc.vector.tensor_scalar_add(
                out=q_sb[:, g, ts * 512:(ts + 1) * 512], in0=ps,
                scalar1=bq_sb[:, g:g + 1])

        def proj_chunk_thunks(ts):
            # stream 512 tokens of x^T; return V/K projection groups as
            # thunks so they can interleave with attention emission
            from functools import partial
            xc = xchunk.tile([P, 8, 512], bf, name="xc", tag="xc")
            dma(out=xc, in_=xk_r[:, :, ts * 512:(ts + 1) * 512])
            th = [partial(_v_group, xc, ts, dhalf, ti)
                  for dhalf in range(2) for ti in range(4)]
            th += [partial(_k_group, xc, ts, g) for g in range(8)]
            return th

        def proj_q_chunk_thunks(ts):
            from functools import partial
            xc = xchunk.tile([P, 8, 512], bf, name="xc", tag="xc")
            dma(out=xc, in_=xq_r[:, :, ts * 512:(ts + 1) * 512])
            return [partial(_q_group, xc, ts, g) for g in range(8)]

        def proj_chunk(ts):
            for t in proj_chunk_thunks(ts):
                t()

        def proj_q_chunk(ts):
            for t in proj_q_chunk_thunks(ts):
                t()

        def attn_slot(g, j):
            # one accumulator bank PER HEAD: PSUM accumulation groups must be
            # bank-aligned on HW (a group at a 256-col offset inside a bank
            # silently corrupts). Row 64 collects the softmax denominator
            # via the ones column of v1_sb.
            pav = [ps_av.tile([65, 256], f32, tag=f"pav{c}",
                              name=f"pav{c}") for c in (0, 1)]
            last_k = 4 * j + 3
            for kk in range(2 * j + 2):  # pairs of tk blocks
                kb = (2 * kk, 2 * kk + 1)
                ps = ps_s.tile([P, 1024], f32, name="scps", tag="scps")
                expp = expps.tile([P, 1024], bf, name="expp", tag="expp")
                for c in (0, 1):         # head within pair
                    for i in (0, 1):     # tk block within pair
                        mm(ps[:, c * 512 + i * 256: c * 512 + i * 256 + 256],
                           lhsT=k_sb[64 * c:64 * c + 64, g,
                                     kb[i] * P:(kb[i] + 1) * P],
                           rhs=q_sb[64 * c:64 * c + 64, g,
                                    j * 256:(j + 1) * 256],
                           start=True, stop=True,
                           tile_position=(64 * c, 0))
                nc.scalar.activation(out=expp, in_=ps, func=EXP, scale=0.125)
                if kk >= 2 * j:
                    nc.vector.tensor_mul(expp, expp, mk_sb[:, kk - 2 * j, :])
                for c in (0, 1):
                    for i in (0, 1):
                        mm(pav[c],
                           lhsT=v1_sb[:, kb[i], 2 * g + c, :],
                           rhs=expp[:, c * 512 + i * 256:
                                    c * 512 + i * 256 + 256],
                           start=(kb[i] == 0), stop=(kb[i] == last_k))
            # copy accumulators to SBUF right away so the PSUM banks free up
            # for the next slot; normalize runs off the PE critical path
            av = [small.tile([65, 256], f32, tag=f"av{c}", bufs=2,
                             name=f"av{c}") for c in (0, 1)]
            for c in (0, 1):
                nc.vector.tensor_copy(out=av[c], in_=pav[c])
            # both heads' denominators into one partition-base-0 tile
            # (reciprocal_approx_fast corrupts base!=0 inputs on HW)
            den2 = small.tile([1, 512], f32, tag="den2", bufs=1, name="den2")
            for c in (0, 1):
                nc.vector.tensor_copy(out=den2[:, c * 256:(c + 1) * 256],
                                      in_=av[c][64:65, :])
            rec = small.tile([1, 512], f32, tag="rec", bufs=1, name="rec")
            nc.vector.reciprocal_approx_fast(out=rec, in_=den2)
            sbb = small.tile([64, 512], f32, tag="sbb", name="sbb")
            nc.gpsimd.partition_broadcast(sbb, rec)
            for c in (0, 1):
                nc.vector.tensor_mul(
                    out=a_sb[64 * c:64 * c + 64, g, j * 256:(j + 1) * 256],
                    in0=av[c][0:64, :], in1=sbb[:, c * 256:(c + 1) * 256])

        def wo_group(ts, o):
            ps = ps_m.tile([P, 512], f32, name="ps", tag="ps")
            for g in range(8):
                mm(ps, lhsT=wo_sb[:, g, o, :],
                   rhs=a_sb[:, g, ts * 512:(ts + 1) * 512],
                   start=(g == 0), stop=(g == 7))
            ysb = small.tile([P, 512], f32, tag="ysb", name="ysb")
            nc.vector.tensor_scalar_add(out=ysb, in0=ps,
                                        scalar1=bo_sb[:, o:o + 1])
            dma(out=y_d[o * P:(o + 1) * P, ts * 512:(ts + 1) * 512],
                in_=ysb)

        def drain(pend, n):
            for _ in range(min(n, len(pend))):
                pend.pop(0)()

        # Emission order sets PE priority: the attention inner loop is paced
        # by the ScalarE exp chain, so feed the PE projection/output-proj
        # groups BETWEEN attention slots to keep it busy (and HAM-warm).
        # attn slot j needs K/V token chunks 0..j and Q chunk j//2;
        # Wo ts=0 needs attention slots 0-1 of all groups, ts=1 slots 2-3.
        proj_chunk(0)
        proj_q_chunk(0)
        proj_chunk(1)
        pend = proj_q_chunk_thunks(1) + proj_chunk_thunks(2)
        for g in range(8):
            attn_slot(g, 0)
            drain(pend, 4)
        drain(pend, 99)
        pend = proj_chunk_thunks(3)
        for g in range(8):
            attn_slot(g, 1)
            drain(pend, 3)
        drain(pend, 99)
        # j=3 before j=2: its longer exp stretch absorbs the Wo ts=0 groups
        pend = []
        for g in range(8):
            attn_slot(g, 3)
            if g == 0:
                pend = [(lambda o=o: wo_group(0, o)) for o in range(8)]
            drain(pend, 1)
        drain(pend, 99)
        for g in range(8):
            attn_slot(g, 2)
        for o in range(8):
            wo_group(1, o)

    nc.compile()
    return nc


def _get_nc():
    if "nc" not in _COMPILED:
        _COMPILED["nc"] = _build_nc()
    return _COMPILED["nc"]


def _core_token_blocks(par):
    return [2 * i + par for i in range(8)]


def _masks(par):
    tri = np.triu(np.ones((P, P), np.float32))   # keep tk <= tq
    on = np.ones((P, P), np.float32)
    z = np.zeros((P, P), np.float32)
    if par == 0:
        m = [np.concatenate(r, axis=1)
             for r in [[tri, on], [z, on], [z, tri], [z, z]]]
    else:
        m = [np.concatenate(r, axis=1)
             for r in [[on, on], [tri, on], [z, on], [z, tri]]]
    # combined group masks matching expp layout [k h0 | k+1 h0 | k h1 | k+1 h1]
    row0 = np.concatenate([m[0], m[1], m[0], m[1]], axis=1)
    row1 = np.concatenate([m[2], m[3], m[2], m[3]], axis=1)
    return np.stack([row0, row1]).astype(BF16)


def _make_in_maps(x, wq, bq, wk, bk, wv, bv, wo, bo):
    bfc = lambda a: np.ascontiguousarray(np.asarray(a, np.float32).T).astype(BF16)
    shared = {
        "wqT": bfc(wq), "wkT": bfc(wk), "wvT": bfc(wv), "woT": bfc(wo),
        "bq_r": np.ascontiguousarray(np.asarray(bq, np.float32).reshape(8, P).T),
        "bk_r": np.ascontiguousarray(np.asarray(bk, np.float32).reshape(8, P).T),
        "bo_r": np.ascontiguousarray(np.asarray(bo, np.float32).reshape(8, P).T),
        "bv_bc": np.ascontiguousarray(
            np.broadcast_to(np.asarray(bv, np.float32).astype(BF16), (P, D))),
    }
    masks = [_masks(0), _masks(1)]
    in_maps, idx_list = [], []
    for core in range(NCORES):
        b, par = core // 2, core % 2
        blocks = _core_token_blocks(par)
        idx = np.concatenate([np.arange(P * blk, P * blk + P) for blk in blocks])
        xT = np.asarray(x[b], np.float32).T
        m = dict(shared)
        m["xk"] = np.ascontiguousarray(xT).astype(BF16)
        m["xq"] = np.ascontiguousarray(xT[:, idx]).astype(BF16)
        m["cmask"] = masks[par]
        in_maps.append(m)
        idx_list.append((b, idx))
    return in_maps, idx_list


def _run(inputs, trace=False):
    from concourse.bass_utils import run_bass_kernel_spmd
    nc = _get_nc()
    in_maps, idx_list = _make_in_maps(**inputs)
    res = run_bass_kernel_spmd(nc, in_maps, list(range(NCORES)), trace=trace)
    y = np.empty((B, T, D), np.float32)
    for core in range(NCORES):
        b, idx = idx_list[core]
        y[b][idx, :] = res.results[core]["yT"].T
    return y, res


def kernel(**inputs):
    y, _ = _run(inputs, trace=False)
    return y


# revision 27
# speedup vs baseline: 1.7289x; 1.0483x over previous
# Multi-head causal attention (B=4, T=2048, D=1024, H=16) on 8 TRN2 NeuronCores.
#
# Sharding: data-parallel over the 4 batches x 2-way split of query rows
# (interleaved 128-row blocks for causal load balance). Every core computes
# K/V for its batch's full sequence (replicated within the batch pair), so
# there are NO collectives. Each core produces the output rows for its own
# 1024 query tokens; the host reassembles.
#
# Per-core kernel (all matmul operands bf16, fp32 PSUM accumulation):
#   qT/kT = W @ x^T (per head-pair group), V kept (t, d)-major with an extra
#   ones column so the attention-value matmul also produces the softmax
#   denominator. Scores are computed transposed (tk partition, tq free),
#   exp on the Scalar engine (no max subtraction: |scores| <= ~3), causal
#   masking via multiplicative {0,1} mask tiles fed per-core, denominator
#   broadcast across partitions on GpSimd.
import numpy as np
import ml_dtypes

B, T, D, H, DH, P = 4, 2048, 1024, 16, 64, 128
NQ = 1024          # query tokens per core
NCORES = 8
BF16 = ml_dtypes.bfloat16

_COMPILED = {}


def _build_nc():
    from contextlib import ExitStack
    import concourse.mybir as mybir
    import concourse.tile as tile
    from concourse import bacc

    bf = mybir.dt.bfloat16
    f32 = mybir.dt.float32
    EXP = mybir.ActivationFunctionType.Exp

    nc = bacc.Bacc("TRN2", target_bir_lowering=False, debug=False,
                   num_devices=NCORES)

    # ---- DRAM I/O ----
    xk_d = nc.dram_tensor("xk", [D, T], bf, kind="ExternalInput").ap()
    xq_d = nc.dram_tensor("xq", [D, NQ], bf, kind="ExternalInput").ap()
    wq_d = nc.dram_tensor("wqT", [D, D], bf, kind="ExternalInput").ap()
    wk_d = nc.dram_tensor("wkT", [D, D], bf, kind="ExternalInput").ap()
    wv_d = nc.dram_tensor("wvT", [D, D], bf, kind="ExternalInput").ap()
    wo_d = nc.dram_tensor("woT", [D, D], bf, kind="ExternalInput").ap()
    bq_d = nc.dram_tensor("bq_r", [P, 8], f32, kind="ExternalInput").ap()
    bk_d = nc.dram_tensor("bk_r", [P, 8], f32, kind="ExternalInput").ap()
    bo_d = nc.dram_tensor("bo_r", [P, 8], f32, kind="ExternalInput").ap()
    bv_d = nc.dram_tensor("bv_bc", [P, D], bf, kind="ExternalInput").ap()
    cm_d = nc.dram_tensor("cmask", [2, P, 1024], bf, kind="ExternalInput").ap()
    y_d = nc.dram_tensor("yT", [D, NQ], f32, kind="ExternalOutput").ap()

    xk_r = xk_d.rearrange("(g p) t -> p g t", p=P)
    xq_r = xq_d.rearrange("(g p) t -> p g t", p=P)

    with tile.TileContext(nc) as tc, ExitStack() as ctx:
        const = ctx.enter_context(tc.tile_pool(name="const", bufs=1))
        xchunk = ctx.enter_context(tc.tile_pool(name="xchunk", bufs=2))
        expps = ctx.enter_context(tc.tile_pool(name="expps", bufs=4))
        small = ctx.enter_context(tc.tile_pool(name="small", bufs=2))
        ps_s = ctx.enter_context(tc.tile_pool(name="ps_s", bufs=2, space="PSUM"))
        ps_av = ctx.enter_context(tc.tile_pool(name="ps_av", bufs=1, space="PSUM"))
        ps_m = ctx.enter_context(tc.tile_pool(name="ps_m", bufs=2, space="PSUM"))

        # ---- resident SBUF tensors ----
        wq_sb = const.tile([P, 8, 8, P], bf)     # [k, kg, g, dout]
        wk_sb = const.tile([P, 8, 8, P], bf)
        wv_sb = const.tile([P, 8, D], bf)        # [k, kg, d] (moving operand)
        wo_sb = const.tile([P, 8, 8, P], bf)     # [d, g, o, dout]
        k_sb = const.tile([P, 8, T], bf)         # k^T per head-pair group
        q_sb = const.tile([P, 8, NQ], bf)
        v1_sb = const.tile([P, 16, 16, 65], bf)  # [tk, tkgrp, head, V|1]
        a_sb = const.tile([P, 8, NQ], bf)        # attention out (d, tq)
        mk_sb = const.tile([P, 2, 1024], bf)
        bq_sb = const.tile([P, 8], f32)
        bk_sb = const.tile([P, 8], f32)
        bo_sb = const.tile([P, 8], f32)
        bv_sb = const.tile([P, D], bf)

        dma = nc.sync.dma_start
        nc.vector.memset(v1_sb[:, :, :, 64:65], 1.0)

        mm = nc.tensor.matmul

        def _v_group(xc, ts, dhalf, ti):
            t = 4 * ts + ti
            ps = ps_m.tile([P, 512], f32, name="ps", tag="ps")
            for kg in range(8):
                mm(ps, lhsT=xc[:, kg, ti * P:(ti + 1) * P],
                   rhs=wv_sb[:, kg, dhalf * 512:(dhalf + 1) * 512],
                   start=(kg == 0), stop=(kg == 7))
            nc.vector.tensor_add(
                out=v1_sb[:, t, 8 * dhalf:8 * dhalf + 8, 0:64],
                in0=ps.rearrange("p (h c) -> p h c", c=64),
                in1=bv_sb[:, dhalf * 512:(dhalf + 1) * 512]
                    .rearrange("p (h c) -> p h c", c=64))

        def _k_group(xc, ts, g):
            ps = ps_m.tile([P, 512], f32, name="ps", tag="ps")
            for kg in range(8):
                mm(ps, lhsT=wk_sb[:, kg, g, :], rhs=xc[:, kg, :],
                   start=(kg == 0), stop=(kg == 7))
            nc.vector.tensor_scalar_add(
                out=k_sb[:, g, ts * 512:(ts + 1) * 512], in0=ps,
                scalar1=bk_sb[:, g:g + 1])

        def _q_group(xc, ts, g):
            ps = ps_m.tile([P, 512], f32, name="ps", tag="ps")
            for kg in range(8):
                mm(ps, lhsT=wq_sb[:, kg, g, :], rhs=xc[:, kg, :],
                   start=(kg == 0), stop=(kg == 7))
            nc.vector.tensor_scalar_add(
                out=q_sb[:, g, ts * 512:(ts + 1) * 512], in0=ps,
                scalar1=bq_sb[:, g:g + 1])

        def proj_chunk_thunks(ts):
            # stream 512 tokens of x^T; return V/K projection groups as
            # thunks so they can interleave with attention emission
            from functools import partial
            xc = xchunk.tile([P, 8, 512], bf, name="xc", tag="xc")
            dma(out=xc, in_=xk_r[:, :, ts * 512:(ts + 1) * 512])
            th = [partial(_v_group, xc, ts, dhalf, ti)
                  for dhalf in range(2) for ti in range(4)]
            th += [partial(_k_group, xc, ts, g) for g in range(8)]
            return th

        def proj_q_chunk_thunks(ts):
            from functools import partial
            xc = xchunk.tile([P, 8, 512], bf, name="xc", tag="xc")
            dma(out=xc, in_=xq_r[:, :, ts * 512:(ts + 1) * 512])
            return [partial(_q_group, xc, ts, g) for g in range(8)]

        def proj_chunk(ts):
            for t in proj_chunk_thunks(ts):
                t()

        def proj_q_chunk(ts):
            for t in proj_q_chunk_thunks(ts):
                t()

        def attn_slot(g, j):
            # one accumulator bank PER HEAD: PSUM accumulation groups must be
            # bank-aligned on HW (a group at a 256-col offset inside a bank
            # silently corrupts). Row 64 collects the softmax denominator
            # via the ones column of v1_sb.
            pav = [ps_av.tile([65, 256], f32, tag=f"pav{c}",
                              name=f"pav{c}") for c in (0, 1)]
            last_k = 4 * j + 3
            for kk in range(2 * j + 2):  # pairs of tk blocks
                kb = (2 * kk, 2 * kk + 1)
                ps = ps_s.tile([P, 1024], f32, name="scps", tag="scps")
                expp = expps.tile([P, 1024], bf, name="expp", tag="expp")
                for c in (0, 1):         # head within pair
                    for i in (0, 1):     # tk block within pair
                        mm(ps[:, c * 512 + i * 256: c * 512 + i * 256 + 256],
                           lhsT=k_sb[64 * c:64 * c + 64, g,
                                     kb[i] * P:(kb[i] + 1) * P],
                           rhs=q_sb[64 * c:64 * c + 64, g,
                                    j * 256:(j + 1) * 256],
                           start=True, stop=True,
                           tile_position=(64 * c, 0))
                nc.scalar.activation(out=expp, in_=ps, func=EXP, scale=0.125)
                if kk >= 2 * j:
                    nc.vector.tensor_mul(expp, expp, mk_sb[:, kk - 2 * j, :])
                for c in (0, 1):
                    for i in (0, 1):
                        mm(pav[c],
                           lhsT=v1_sb[:, kb[i], 2 * g + c, :],
                           rhs=expp[:, c * 512 + i * 256:
                                    c * 512 + i * 256 + 256],
                           start=(kb[i] == 0), stop=(kb[i] == last_k))
            # copy accumulators to SBUF right away so the PSUM banks free up
            # for the next slot; normalize runs off the PE critical path
            av = [small.tile([65, 256], f32, tag=f"av{c}", bufs=2,
                             name=f"av{c}") for c in (0, 1)]
            for c in (0, 1):
                nc.vector.tensor_copy(out=av[c], in_=pav[c])
            # both heads' denominators into one partition-base-0 tile
            # (reciprocal_approx_fast corrupts base!=0 inputs on HW)
            den2 = small.tile([1, 512], f32, tag="den2", bufs=1, name="den2")
            for c in (0, 1):
                nc.vector.tensor_copy(out=den2[:, c * 256:(c + 1) * 256],
                                      in_=av[c][64:65, :])
            rec = small.tile([1, 512], f32, tag="rec", bufs=1, name="rec")
            nc.vector.reciprocal_approx_fast(out=rec, in_=den2)
            sbb = small.tile([64, 512], f32, tag="sbb", name="sbb")
            nc.gpsimd.partition_broadcast(sbb, rec)
            for c in (0, 1):
                nc.vector.tensor_mul(
                    out=a_sb[64 * c:64 * c + 64, g, j * 256:(j + 1) * 256],
                    in0=av[c][0:64, :], in1=sbb[:, c * 256:(c + 1) * 256])

        def wo_group(q4, o):
            # 256-wide output-projection chunk: needs only attention slot q4
            ps = ps_m.tile([P, 512], f32, name="ps", tag="ps")
            for g in range(8):
                mm(ps[:, 0:256], lhsT=wo_sb[:, g, o, :],
                   rhs=a_sb[:, g, q4 * 256:(q4 + 1) * 256],
                   start=(g == 0), stop=(g == 7))
            ysb = small.tile([P, 512], f32, tag="ysb", name="ysb")
            nc.vector.tensor_scalar_add(out=ysb[:, 0:256], in0=ps[:, 0:256],
                                        scalar1=bo_sb[:, o:o + 1])
            dma(out=y_d[o * P:(o + 1) * P, q4 * 256:(q4 + 1) * 256],
                in_=ysb[:, 0:256])

        def drain(pend, n):
            for _ in range(min(n, len(pend))):
                pend.pop(0)()

        # Emission order sets PE priority: the attention inner loop is paced
        # by the ScalarE exp chain, so feed the PE projection/output-proj
        # groups BETWEEN attention slots to keep it busy (and HAM-warm).
        # Interleave the initial weight/x DMAs with first-chunk compute so
        # the PE isn't parked behind serialized input loads.
        dma(out=bv_sb, in_=bv_d)
        dma(out=wv_sb, in_=wv_d.rearrange("(kg p) d -> p kg d", p=P))
        pend0 = proj_chunk_thunks(0)           # fires the xk chunk-0 DMA
        dma(out=wk_sb, in_=wk_d.rearrange("(kg p) (g c) -> p kg g c", p=P, c=P))
        dma(out=bk_sb, in_=bk_d)
        dma(out=bq_sb, in_=bq_d)
        drain(pend0, 8)                        # V-proj of chunk 0
        dma(out=wq_sb, in_=wq_d.rearrange("(kg p) (g c) -> p kg g c", p=P, c=P))
        pq0 = proj_q_chunk_thunks(0)
        drain(pend0, 99)                       # K-proj of chunk 0
        dma(out=mk_sb, in_=cm_d.rearrange("m p c -> p m c"))
        dma(out=bo_sb, in_=bo_d)
        p1 = proj_chunk_thunks(1)
        drain(pq0, 99)
        drain(p1, 99)
        dma(out=wo_sb, in_=wo_d.rearrange("(g p) (o c) -> p g o c", p=P, c=P))

        # attn slot j needs K/V token chunks 0..j and Q chunk j//2;
        # Wo chunk q4 needs attention slot q4 of all groups.
        pend = proj_q_chunk_thunks(1) + proj_chunk_thunks(2)
        for g in range(8):
            attn_slot(g, 0)
            drain(pend, 4)
        drain(pend, 99)
        pend = proj_chunk_thunks(3) + [(lambda o=o: wo_group(0, o))
                                       for o in range(8)]
        for g in range(8):
            attn_slot(g, 1)
            drain(pend, 4)
        drain(pend, 99)
        # j=3 before j=2 so Wo chunks 1 and 3 both get an exp stretch to hide in
        pend = [(lambda o=o: wo_group(1, o)) for o in range(8)]
        for g in range(8):
            attn_slot(g, 3)
            drain(pend, 1)
        drain(pend, 99)
        pend = [(lambda o=o: wo_group(3, o)) for o in range(8)]
        for g in range(8):
            attn_slot(g, 2)
            drain(pend, 1)
        drain(pend, 99)
        for o in range(8):
            wo_group(2, o)

    nc.compile()
    return nc


def _get_nc():
    if "nc" not in _COMPILED:
        _COMPILED["nc"] = _build_nc()
    return _COMPILED["nc"]


def _core_token_blocks(par):
    return [2 * i + par for i in range(8)]


def _masks(par):
    tri = np.triu(np.ones((P, P), np.float32))   # keep tk <= tq
    on = np.ones((P, P), np.float32)
    z = np.zeros((P, P), np.float32)
    if par == 0:
        m = [np.concatenate(r, axis=1)
             for r in [[tri, on], [z, on], [z, tri], [z, z]]]
    else:
        m = [np.concatenate(r, axis=1)
             for r in [[on, on], [tri, on], [z, on], [z, tri]]]
    # combined group masks matching expp layout [k h0 | k+1 h0 | k h1 | k+1 h1]
    row0 = np.concatenate([m[0], m[1], m[0], m[1]], axis=1)
    row1 = np.concatenate([m[2], m[3], m[2], m[3]], axis=1)
    return np.stack([row0, row1]).astype(BF16)


def _make_in_maps(x, wq, bq, wk, bk, wv, bv, wo, bo):
    bfc = lambda a: np.ascontiguousarray(np.asarray(a, np.float32).T).astype(BF16)
    shared = {
        "wqT": bfc(wq), "wkT": bfc(wk), "wvT": bfc(wv), "woT": bfc(wo),
        "bq_r": np.ascontiguousarray(np.asarray(bq, np.float32).reshape(8, P).T),
        "bk_r": np.ascontiguousarray(np.asarray(bk, np.float32).reshape(8, P).T),
        "bo_r": np.ascontiguousarray(np.asarray(bo, np.float32).reshape(8, P).T),
        "bv_bc": np.ascontiguousarray(
            np.broadcast_to(np.asarray(bv, np.float32).astype(BF16), (P, D))),
    }
    masks = [_masks(0), _masks(1)]
    in_maps, idx_list = [], []
    for core in range(NCORES):
        b, par = core // 2, core % 2
        blocks = _core_token_blocks(par)
        idx = np.concatenate([np.arange(P * blk, P * blk + P) for blk in blocks])
        xT = np.asarray(x[b], np.float32).T
        m = dict(shared)
        m["xk"] = np.ascontiguousarray(xT).astype(BF16)
        m["xq"] = np.ascontiguousarray(xT[:, idx]).astype(BF16)
        m["cmask"] = masks[par]
        in_maps.append(m)
        idx_list.append((b, idx))
    return in_maps, idx_list


def _run(inputs, trace=False):
    from concourse.bass_utils import run_bass_kernel_spmd
    nc = _get_nc()
    in_maps, idx_list = _make_in_maps(**inputs)
    res = run_bass_kernel_spmd(nc, in_maps, list(range(NCORES)), trace=trace)
    y = np.empty((B, T, D), np.float32)
    for core in range(NCORES):
        b, idx = idx_list[core]
        y[b][idx, :] = res.results[core]["yT"].T
    return y, res


def kernel(**inputs):
    y, _ = _run(inputs, trace=False)
    return y


# revision 28
# speedup vs baseline: 1.7332x; 1.0025x over previous
# Multi-head causal attention (B=4, T=2048, D=1024, H=16) on 8 TRN2 NeuronCores.
#
# Sharding: data-parallel over the 4 batches x 2-way split of query rows
# (interleaved 128-row blocks for causal load balance). Every core computes
# K/V for its batch's full sequence (replicated within the batch pair), so
# there are NO collectives. Each core produces the output rows for its own
# 1024 query tokens; the host reassembles.
#
# Per-core kernel (all matmul operands bf16, fp32 PSUM accumulation):
#   qT/kT = W @ x^T (per head-pair group), V kept (t, d)-major with an extra
#   ones column so the attention-value matmul also produces the softmax
#   denominator. Scores are computed transposed (tk partition, tq free),
#   exp on the Scalar engine (no max subtraction: |scores| <= ~3), causal
#   masking via multiplicative {0,1} mask tiles fed per-core, denominator
#   broadcast across partitions on GpSimd.
import numpy as np
import ml_dtypes

B, T, D, H, DH, P = 4, 2048, 1024, 16, 64, 128
NQ = 1024          # query tokens per core
NCORES = 8
BF16 = ml_dtypes.bfloat16

_COMPILED = {}


def _build_nc():
    from contextlib import ExitStack
    import concourse.mybir as mybir
    import concourse.tile as tile
    from concourse import bacc

    bf = mybir.dt.bfloat16
    f32 = mybir.dt.float32
    EXP = mybir.ActivationFunctionType.Exp

    nc = bacc.Bacc("TRN2", target_bir_lowering=False, debug=False,
                   num_devices=NCORES)

    # ---- DRAM I/O ----
    xk_d = nc.dram_tensor("xk", [D, T], bf, kind="ExternalInput").ap()
    xq_d = nc.dram_tensor("xq", [D, NQ], bf, kind="ExternalInput").ap()
    wq_d = nc.dram_tensor("wqT", [D, D], bf, kind="ExternalInput").ap()
    wk_d = nc.dram_tensor("wkT", [D, D], bf, kind="ExternalInput").ap()
    wv_d = nc.dram_tensor("wvT", [D, D], bf, kind="ExternalInput").ap()
    wo_d = nc.dram_tensor("woT", [D, D], bf, kind="ExternalInput").ap()
    bq_d = nc.dram_tensor("bq_r", [P, 8], f32, kind="ExternalInput").ap()
    bk_d = nc.dram_tensor("bk_r", [P, 8], f32, kind="ExternalInput").ap()
    bo_d = nc.dram_tensor("bo_r", [P, 8], f32, kind="ExternalInput").ap()
    bv_d = nc.dram_tensor("bv_bc", [P, D], bf, kind="ExternalInput").ap()
    cm_d = nc.dram_tensor("cmask", [2, P, 1024], bf, kind="ExternalInput").ap()
    y_d = nc.dram_tensor("yT", [D, NQ], f32, kind="ExternalOutput").ap()

    xk_r = xk_d.rearrange("(g p) t -> p g t", p=P)
    xq_r = xq_d.rearrange("(g p) t -> p g t", p=P)

    with tile.TileContext(nc) as tc, ExitStack() as ctx:
        const = ctx.enter_context(tc.tile_pool(name="const", bufs=1))
        xchunk = ctx.enter_context(tc.tile_pool(name="xchunk", bufs=2))
        expps = ctx.enter_context(tc.tile_pool(name="expps", bufs=4))
        small = ctx.enter_context(tc.tile_pool(name="small", bufs=2))
        ps_s = ctx.enter_context(tc.tile_pool(name="ps_s", bufs=2, space="PSUM"))
        ps_av = ctx.enter_context(tc.tile_pool(name="ps_av", bufs=1, space="PSUM"))
        ps_m = ctx.enter_context(tc.tile_pool(name="ps_m", bufs=2, space="PSUM"))

        # ---- resident SBUF tensors ----
        wq_sb = const.tile([P, 8, 8, P], bf)     # [k, kg, g, dout]
        wk_sb = const.tile([P, 8, 8, P], bf)
        wv_sb = const.tile([P, 8, D], bf)        # [k, kg, d] (moving operand)
        wo_sb = const.tile([P, 8, 8, P], bf)     # [d, g, o, dout]
        k_sb = const.tile([P, 8, T], bf)         # k^T per head-pair group
        q_sb = const.tile([P, 8, NQ], bf)
        v1_sb = const.tile([P, 16, 16, 65], bf)  # [tk, tkgrp, head, V|1]
        a_sb = const.tile([P, 8, NQ], bf)        # attention out (d, tq)
        mk_sb = const.tile([P, 2, 1024], bf)
        bq_sb = const.tile([P, 8], f32)
        bk_sb = const.tile([P, 8], f32)
        bo_sb = const.tile([P, 8], f32)
        bv_sb = const.tile([P, D], bf)

        dma = nc.sync.dma_start
        nc.vector.memset(v1_sb[:, :, :, 64:65], 1.0)

        mm = nc.tensor.matmul

        def _v_group(xc, ts, dhalf, ti):
            t = 4 * ts + ti
            ps = ps_m.tile([P, 512], f32, name="ps", tag="ps")
            for kg in range(8):
                mm(ps, lhsT=xc[:, kg, ti * P:(ti + 1) * P],
                   rhs=wv_sb[:, kg, dhalf * 512:(dhalf + 1) * 512],
                   start=(kg == 0), stop=(kg == 7))
            nc.vector.tensor_add(
                out=v1_sb[:, t, 8 * dhalf:8 * dhalf + 8, 0:64],
                in0=ps.rearrange("p (h c) -> p h c", c=64),
                in1=bv_sb[:, dhalf * 512:(dhalf + 1) * 512]
                    .rearrange("p (h c) -> p h c", c=64))

        def _k_group(xc, ts, g):
            ps = ps_m.tile([P, 512], f32, name="ps", tag="ps")
            for kg in range(8):
                mm(ps, lhsT=wk_sb[:, kg, g, :], rhs=xc[:, kg, :],
                   start=(kg == 0), stop=(kg == 7))
            nc.vector.tensor_scalar_add(
                out=k_sb[:, g, ts * 512:(ts + 1) * 512], in0=ps,
                scalar1=bk_sb[:, g:g + 1])

        def _q_group(xc, ts, g):
            ps = ps_m.tile([P, 512], f32, name="ps", tag="ps")
            for kg in range(8):
                mm(ps, lhsT=wq_sb[:, kg, g, :], rhs=xc[:, kg, :],
                   start=(kg == 0), stop=(kg == 7))
            nc.vector.tensor_scalar_add(
                out=q_sb[:, g, ts * 512:(ts + 1) * 512], in0=ps,
                scalar1=bq_sb[:, g:g + 1])

        def proj_chunk_thunks(ts):
            # stream 512 tokens of x^T; return V/K projection groups as
            # thunks so they can interleave with attention emission
            from functools import partial
            xc = xchunk.tile([P, 8, 512], bf, name="xc", tag="xc")
            dma(out=xc, in_=xk_r[:, :, ts * 512:(ts + 1) * 512])
            th = [partial(_v_group, xc, ts, dhalf, ti)
                  for dhalf in range(2) for ti in range(4)]
            th += [partial(_k_group, xc, ts, g) for g in range(8)]
            return th

        def proj_q_chunk_thunks(ts):
            from functools import partial
            xc = xchunk.tile([P, 8, 512], bf, name="xc", tag="xc")
            dma(out=xc, in_=xq_r[:, :, ts * 512:(ts + 1) * 512])
            return [partial(_q_group, xc, ts, g) for g in range(8)]

        def proj_chunk(ts):
            for t in proj_chunk_thunks(ts):
                t()

        def proj_q_chunk(ts):
            for t in proj_q_chunk_thunks(ts):
                t()

        def attn_slot(g, j):
            # one accumulator bank PER HEAD: PSUM accumulation groups must be
            # bank-aligned on HW (a group at a 256-col offset inside a bank
            # silently corrupts). Row 64 collects the softmax denominator
            # via the ones column of v1_sb.
            pav = [ps_av.tile([65, 256], f32, tag=f"pav{c}",
                              name=f"pav{c}") for c in (0, 1)]
            last_k = 4 * j + 3
            for kk in range(2 * j + 2):  # pairs of tk blocks
                kb = (2 * kk, 2 * kk + 1)
                ps = ps_s.tile([P, 1024], f32, name="scps", tag="scps")
                expp = expps.tile([P, 1024], bf, name="expp", tag="expp")
                for c in (0, 1):         # head within pair
                    for i in (0, 1):     # tk block within pair
                        mm(ps[:, c * 512 + i * 256: c * 512 + i * 256 + 256],
                           lhsT=k_sb[64 * c:64 * c + 64, g,
                                     kb[i] * P:(kb[i] + 1) * P],
                           rhs=q_sb[64 * c:64 * c + 64, g,
                                    j * 256:(j + 1) * 256],
                           start=True, stop=True,
                           tile_position=(64 * c, 0))
                nc.scalar.activation(out=expp, in_=ps, func=EXP, scale=0.125)
                if kk >= 2 * j:
                    nc.vector.tensor_mul(expp, expp, mk_sb[:, kk - 2 * j, :])
                for c in (0, 1):
                    for i in (0, 1):
                        mm(pav[c],
                           lhsT=v1_sb[:, kb[i], 2 * g + c, :],
                           rhs=expp[:, c * 512 + i * 256:
                                    c * 512 + i * 256 + 256],
                           start=(kb[i] == 0), stop=(kb[i] == last_k))
            # copy accumulators to SBUF right away so the PSUM banks free up
            # for the next slot; normalize runs off the PE critical path
            av = [small.tile([65, 256], f32, tag=f"av{c}", bufs=2,
                             name=f"av{c}") for c in (0, 1)]
            for c in (0, 1):
                nc.vector.tensor_copy(out=av[c], in_=pav[c])
            # both heads' denominators into one partition-base-0 tile
            # (reciprocal_approx_fast corrupts base!=0 inputs on HW)
            den2 = small.tile([1, 512], f32, tag="den2", bufs=1, name="den2")
            for c in (0, 1):
                nc.vector.tensor_copy(out=den2[:, c * 256:(c + 1) * 256],
                                      in_=av[c][64:65, :])
            rec = small.tile([1, 512], f32, tag="rec", bufs=1, name="rec")
            nc.vector.reciprocal_approx_fast(out=rec, in_=den2)
            sbb = small.tile([64, 512], f32, tag="sbb", name="sbb")
            nc.gpsimd.partition_broadcast(sbb, rec)
            for c in (0, 1):
                nc.vector.tensor_mul(
                    out=a_sb[64 * c:64 * c + 64, g, j * 256:(j + 1) * 256],
                    in0=av[c][0:64, :], in1=sbb[:, c * 256:(c + 1) * 256])

        def wo_group(q4, o):
            # 256-wide output-projection chunk: needs only attention slot q4
            ps = ps_m.tile([P, 512], f32, name="ps", tag="ps")
            for g in range(8):
                mm(ps[:, 0:256], lhsT=wo_sb[:, g, o, :],
                   rhs=a_sb[:, g, q4 * 256:(q4 + 1) * 256],
                   start=(g == 0), stop=(g == 7))
            ysb = small.tile([P, 512], f32, tag="ysb", name="ysb")
            nc.vector.tensor_scalar_add(out=ysb[:, 0:256], in0=ps[:, 0:256],
                                        scalar1=bo_sb[:, o:o + 1])
            dma(out=y_d[o * P:(o + 1) * P, q4 * 256:(q4 + 1) * 256],
                in_=ysb[:, 0:256])

        def drain(pend, n):
            for _ in range(min(n, len(pend))):
                pend.pop(0)()

        # Emission order sets PE priority: the attention inner loop is paced
        # by the ScalarE exp chain, so feed the PE projection/output-proj
        # groups BETWEEN attention slots to keep it busy (and HAM-warm).
        # Interleave the initial weight/x DMAs with first-chunk compute so
        # the PE isn't parked behind serialized input loads.
        dma(out=bv_sb, in_=bv_d)
        dma(out=wv_sb, in_=wv_d.rearrange("(kg p) d -> p kg d", p=P))
        pend0 = proj_chunk_thunks(0)           # fires the xk chunk-0 DMA
        dma(out=wk_sb, in_=wk_d.rearrange("(kg p) (g c) -> p kg g c", p=P, c=P))
        dma(out=bk_sb, in_=bk_d)
        dma(out=bq_sb, in_=bq_d)
        drain(pend0, 8)                        # V-proj of chunk 0
        dma(out=wq_sb, in_=wq_d.rearrange("(kg p) (g c) -> p kg g c", p=P, c=P))
        pq0 = proj_q_chunk_thunks(0)
        drain(pend0, 99)                       # K-proj of chunk 0
        dma(out=mk_sb, in_=cm_d.rearrange("m p c -> p m c"))
        dma(out=bo_sb, in_=bo_d)
        p1 = proj_chunk_thunks(1)
        drain(pq0, 99)
        dma(out=wo_sb, in_=wo_d.rearrange("(g p) (o c) -> p g o c", p=P, c=P))

        # attn slot j needs K/V token chunks 0..j and Q chunk j//2 (all of
        # slot 0's inputs complete with chunk 0), so the chunk-1/2 projection
        # groups interleave into the j=0 attention loop.
        # Wo chunk q4 needs attention slot q4 of all groups.
        pend = p1 + proj_q_chunk_thunks(1) + proj_chunk_thunks(2)
        for g in range(8):
            attn_slot(g, 0)
            drain(pend, 7)
        drain(pend, 99)
        pend = proj_chunk_thunks(3) + [(lambda o=o: wo_group(0, o))
                                       for o in range(8)]
        for g in range(8):
            attn_slot(g, 1)
            drain(pend, 4)
        drain(pend, 99)
        # j=3 before j=2 so Wo chunks 1 and 3 both get an exp stretch to hide in
        pend = [(lambda o=o: wo_group(1, o)) for o in range(8)]
        for g in range(8):
            attn_slot(g, 3)
            drain(pend, 1)
        drain(pend, 99)
        pend = [(lambda o=o: wo_group(3, o)) for o in range(8)]
        for g in range(8):
            attn_slot(g, 2)
            drain(pend, 1)
        drain(pend, 99)
        for o in range(8):
            wo_group(2, o)

    nc.compile()
    return nc


def _get_nc():
    if "nc" not in _COMPILED:
        _COMPILED["nc"] = _build_nc()
    return _COMPILED["nc"]


def _core_token_blocks(par):
    return [2 * i + par for i in range(8)]


def _masks(par):
    tri = np.triu(np.ones((P, P), np.float32))   # keep tk <= tq
    on = np.ones((P, P), np.float32)
    z = np.zeros((P, P), np.float32)
    if par == 0:
        m = [np.concatenate(r, axis=1)
             for r in [[tri, on], [z, on], [z, tri], [z, z]]]
    else:
        m = [np.concatenate(r, axis=1)
             for r in [[on, on], [tri, on], [z, on], [z, tri]]]
    # combined group masks matching expp layout [k h0 | k+1 h0 | k h1 | k+1 h1]
    row0 = np.concatenate([m[0], m[1], m[0], m[1]], axis=1)
    row1 = np.concatenate([m[2], m[3], m[2], m[3]], axis=1)
    return np.stack([row0, row1]).astype(BF16)


def _make_in_maps(x, wq, bq, wk, bk, wv, bv, wo, bo):
    bfc = lambda a: np.ascontiguousarray(np.asarray(a, np.float32).T).astype(BF16)
    shared = {
        "wqT": bfc(wq), "wkT": bfc(wk), "wvT": bfc(wv), "woT": bfc(wo),
        "bq_r": np.ascontiguousarray(np.asarray(bq, np.float32).reshape(8, P).T),
        "bk_r": np.ascontiguousarray(np.asarray(bk, np.float32).reshape(8, P).T),
        "bo_r": np.ascontiguousarray(np.asarray(bo, np.float32).reshape(8, P).T),
        "bv_bc": np.ascontiguousarray(
            np.broadcast_to(np.asarray(bv, np.float32).astype(BF16), (P, D))),
    }
    masks = [_masks(0), _masks(1)]
    in_maps, idx_list = [], []
    for core in range(NCORES):
        b, par = core // 2, core % 2
        blocks = _core_token_blocks(par)
        idx = np.concatenate([np.arange(P * blk, P * blk + P) for blk in blocks])
        xT = np.asarray(x[b], np.float32).T
        m = dict(shared)
        m["xk"] = np.ascontiguousarray(xT).astype(BF16)
        m["xq"] = np.ascontiguousarray(xT[:, idx]).astype(BF16)
        m["cmask"] = masks[par]
        in_maps.append(m)
        idx_list.append((b, idx))
    return in_maps, idx_list


def _run(inputs, trace=False):
    from concourse.bass_utils import run_bass_kernel_spmd
    nc = _get_nc()
    in_maps, idx_list = _make_in_maps(**inputs)
    res = run_bass_kernel_spmd(nc, in_maps, list(range(NCORES)), trace=trace)
    y = np.empty((B, T, D), np.float32)
    for core in range(NCORES):
        b, idx = idx_list[core]
        y[b][idx, :] = res.results[core]["yT"].T
    return y, res


def kernel(**inputs):
    y, _ = _run(inputs, trace=False)
    return y
